# revision 32
# baseline (speedup 1.0000x reference)
"""Trainium2 Bass kernel for nn_BatchDifferentiableKF (v3).

Problem: batched 4-state Kalman filter, B=16384 rows, T=512 steps,
state [px, py, vx, vy], measurements = predicted velocities (B, T, 2).

Structure exploited:
  * Gains are data-independent -> fixed schedule computed on host.
  * x/y channels decouple into two identical scalar filters, LINEAR in
    (z, p0):  p_t = p_{t-1} + g[t] v_{t-1} + k_p[t] z_t ;
              v_t = a[t] v_{t-1} + k_v[t] z_t.
  * Chunk T into 4 x 128: per chunk the map (carry, z) -> outputs is a
    dense lower-triangular matrix pair plus rank-2 carry terms; chunks
    1..3 share identical steady-state weights.

v3 design notes (vs the first working version):
  * z arrives TIME-MAJOR from the host (z_tm [1024, B]); the 144
    on-device PE transposes, their PSUM evacuations, and the fp32->bf16
    DVE cast pass are all gone.  Input tiles load with a casting SWDGE
    DMA (f32 HBM -> bf16 SBUF).
  * Matmuls stay BIG (N=512/320/512 per output tile): many small
    bank-cycling matmuls keep the PE HAM clock gate cold (measured:
    every matmul at the (219+N)/1.2 ns cold-isolated cost).
  * All constants in 3 packed DMAs; PE warm-up runs on a memset tile
    from t~0 so the clock gate opens before real work arrives.
  * Initial carries come from a host-built (p0x, p0y, 0, 0) array --
    no per-tile p0 loads/transposes.
  * Batch groups shrink toward the end (4,4,4,2,1,1 tiles) and outputs
    DMA per-tile right after their last chunk -> short drain tail.

Sharding: embarrassingly parallel over batch across the 8 cores.
"""

import numpy as np
import ml_dtypes

B_FULL = 16384
T = 512
C = 128          # chunk length
NCH = T // C     # 4 chunks
N_CORES = 8
B_CORE = B_FULL // N_CORES   # 2048
VEL_KEEP = 16    # vel carry columns kept (a_ss^16 ~ 7e-10)
CW_N = 2 * C + 2 * VEL_KEEP  # 320 carry-matmul columns
GROUPS = [4, 4, 4, 2, 1, 1]  # batch tiles per carry group

bf16 = ml_dtypes.bfloat16


# ----------------------------------------------------------------------------
# Host-side weight construction (float64)
# ----------------------------------------------------------------------------

def _gains(dt, q_pos, q_vel, r_vel, n):
    """Gain schedule k_p[t], k_v[t] of the decoupled scalar filter, P0=I."""
    dt = float(np.float32(dt))
    r = float(np.float32(r_vel)) + float(np.float32(1e-6))
    qp = float(np.float32(q_pos))
    qv = float(np.float32(q_vel))
    Ppp, Ppv, Pvv = 1.0, 0.0, 1.0
    k_p = np.zeros(n)
    k_v = np.zeros(n)
    for t in range(n):
        Ppv_ = Ppv + dt * Pvv
        Ppp_ = Ppp + 2.0 * dt * Ppv + dt * dt * Pvv + qp
        Pvv_ = Pvv + qv
        S = Pvv_ + r
        k_p[t] = Ppv_ / S
        k_v[t] = Pvv_ / S
        Ppp = Ppp_ - k_p[t] * Ppv_
        Ppv = Ppv_ - k_p[t] * Pvv_
        Pvv = Pvv_ - k_v[t] * Pvv_
    return k_p, k_v


def _chunk_maps(k_p, k_v, dt):
    """Per-chunk affine maps: (p_in, v_in, z[0..C-1]) -> (p[..], v[..]).

    p_out[i] = p_in + Bv[m][i] v_in + sum_j Wp[m][i,j] z[j]
    v_out[i] =        Av[m][i] v_in + sum_j Wv[m][i,j] z[j]
    """
    g = dt - k_p
    a = 1.0 - k_v
    Wp = np.zeros((NCH, C, C))
    Wv = np.zeros((NCH, C, C))
    Av = np.zeros((NCH, C))
    Bv = np.zeros((NCH, C))
    for m in range(NCH):
        pcoef = np.zeros(C + 1)
        vcoef = np.zeros(C + 1)
        vcoef[0] = 1.0
        for i in range(C):
            t = m * C + i
            pcoef = pcoef + g[t] * vcoef
            pcoef[1 + i] += k_p[t]
            vcoef = a[t] * vcoef
            vcoef[1 + i] += k_v[t]
            Bv[m, i] = pcoef[0]
            Wp[m, i] = pcoef[1:]
            Av[m, i] = vcoef[0]
            Wv[m, i] = vcoef[1:]
    return Wp, Wv, Av, Bv


def build_weights(dt, q_pos, q_vel, r_vel):
    """Device constants. Data layout (matches v1):

    zt row q of chunk-half tile k <-> (s = 64k + q//2, c = q&1);
    output free index f = pv*256 + t*2 + c.
    Carry rows e: 0,1 = p_in (c=0,1); 2,3 = v_in (c=0,1).

    wpack [128, 2064] bf16: 4 x wmain[mset][h] (512 cols each) at
      [512*(2*mset+h)] | 4 x bw[mset][h] (4 cols each) at [2048+...].
    cwpack [4, 2*CW_N+8] bf16: cw[mset] (CW_N cols) at [CW_N*mset] |
      mw[mset] (4 cols) at [2*CW_N+4*mset].
    """
    dtf = float(np.float32(dt))
    k_p, k_v = _gains(dt, q_pos, q_vel, r_vel, T)
    Wp, Wv, Av, Bv = _chunk_maps(k_p, k_v, dtf)

    wpack = np.zeros((128, 2064))
    cwpack = np.zeros((4, 2 * CW_N + 8))
    for mset in range(2):
        for h in range(2):
            w = np.zeros((128, 512))
            bw = np.zeros((128, 4))
            for q in range(128):
                j = 64 * h + q // 2
                c = q & 1
                w[q, 0 * 256 + 2 * np.arange(C) + c] = Wp[mset, :, j]
                w[q, 1 * 256 + 2 * np.arange(C) + c] = Wv[mset, :, j]
                bw[q, c] = Wp[mset, C - 1, j]
                bw[q, 2 + c] = Wv[mset, C - 1, j]
            wpack[:, 512 * (2 * mset + h):512 * (2 * mset + h + 1)] = w
            wpack[:, 2048 + 4 * (2 * mset + h):2048 + 4 * (2 * mset + h + 1)] = bw
        cw = np.zeros((4, CW_N))
        for cp in range(2):
            cw[cp, 2 * np.arange(C) + cp] = 1.0
            cw[2 + cp, 2 * np.arange(C) + cp] = Bv[mset]
            cw[2 + cp, 2 * C + 2 * np.arange(VEL_KEEP) + cp] = Av[mset, :VEL_KEEP]
        cwpack[:, CW_N * mset:CW_N * (mset + 1)] = cw
        mw = np.zeros((4, 4))
        for cp in range(2):
            mw[cp, cp] = 1.0
            mw[2 + cp, cp] = Bv[mset, C - 1]
            mw[2 + cp, 2 + cp] = Av[mset, C - 1]
        cwpack[:, 2 * CW_N + 4 * mset:2 * CW_N + 4 * (mset + 1)] = mw
    return {"wpack": wpack.astype(bf16), "cwpack": cwpack.astype(bf16)}


# ----------------------------------------------------------------------------
# Bass kernel
# ----------------------------------------------------------------------------

def build_nc(n_bt):
    """Build the Bass program for one core processing n_bt*128 batch rows."""
    import concourse.bass as bass
    import concourse.tile as tile
    from concourse import bacc, mybir
    from contextlib import ExitStack

    f32 = mybir.dt.float32
    bf = mybir.dt.bfloat16

    b_sz = n_bt * 128
    assert sum(GROUPS) == n_bt
    gstart = np.cumsum([0] + GROUPS[:-1])
    nc = bacc.Bacc("TRN2", target_bir_lowering=False, debug=False)

    z_tm = nc.dram_tensor("z_tm", [1024, b_sz], f32, kind="ExternalInput").ap()
    p0z_in = nc.dram_tensor("p0z_in", [4, b_sz], bf, kind="ExternalInput").ap()
    wpack_d = nc.dram_tensor("wpack", [128, 2064], bf,
                             kind="ExternalInput").ap()
    cwpack_d = nc.dram_tensor("cwpack", [4, 2 * CW_N + 8], bf,
                              kind="ExternalInput").ap()
    pos_out = nc.dram_tensor("pos_out", [b_sz, 1024], f32,
                             kind="ExternalOutput").ap()
    vel_out = nc.dram_tensor("vel_out", [b_sz, 1024], f32,
                             kind="ExternalOutput").ap()

    with tile.TileContext(nc) as tc, ExitStack() as ctx:
        const = ctx.enter_context(tc.tile_pool(name="const", bufs=1))
        ztp = ctx.enter_context(tc.tile_pool(name="ztp", bufs=1))
        ktp = ctx.enter_context(tc.tile_pool(name="ktp", bufs=1))
        stage = ctx.enter_context(tc.tile_pool(name="stage", bufs=2))
        ps_main = ctx.enter_context(tc.tile_pool(name="ps_main", bufs=6,
                                                 space="PSUM"))
        ps_c = ctx.enter_context(tc.tile_pool(name="ps_c", bufs=2,
                                              space="PSUM"))

        # ---- constants ----
        wsb = const.tile([128, 2064], bf, name="wsb", tag="wsb")
        cwsb = const.tile([4, 2 * CW_N + 8], bf, name="cwsb", tag="cwsb")
        p0sb = const.tile([4, b_sz], bf, name="p0sb", tag="p0sb")
        warmw = const.tile([128, 512], bf, name="warmw", tag="warmw")
        nc.scalar.dma_start(wsb[:], wpack_d)
        nc.scalar.dma_start(cwsb[:], cwpack_d)
        nc.scalar.dma_start(p0sb[:], p0z_in)

        def wmain(ms, h):
            return wsb[:, 512 * (2 * ms + h):512 * (2 * ms + h + 1)]

        def bwv(ms, h):
            o = 2048 + 4 * (2 * ms + h)
            return wsb[:, o:o + 4]

        def cwm(ms):
            return cwsb[:, CW_N * ms:CW_N * (ms + 1)]

        def mwm(ms):
            return cwsb[:, 2 * CW_N + 4 * ms:2 * CW_N + 4 * (ms + 1)]

        # ---- PE warm-up on a memset tile: no DMA dependency. Sized to
        # keep the PE busy until the first input tiles land, so the HAM
        # K=8/8 state survives into the real matmul stream. ----
        nc.vector.memset(warmw[:], 0.03125)
        warm_ps = ps_main.tile([128, 512], f32, tag="out")
        for wi in range(22):
            nc.tensor.matmul(warm_ps[:], warmw[:, 0:128], warmw[:],
                             start=(wi == 0), stop=(wi == 21))

        def warm_burst(n):
            """Dependency-free burst to re-open the HAM clock gate if an
            input stall re-throttled the PE."""
            wps = ps_main.tile([128, 512], f32, tag="out")
            for wi in range(n):
                nc.tensor.matmul(wps[:], warmw[:, 0:128], warmw[:],
                                 start=(wi == 0), stop=(wi == n - 1))

        # ---- input: zt[k] [128 (s,c), b] bf16, cast during DMA.
        # Quarter-column slices, need-ordered: the first block's two
        # tiles are the first (short) DMAs in the queue, so compute
        # starts early despite SDMA round-robin across queued DMAs. ----
        zt = [ztp.tile([128, b_sz], bf, name=f"zt_{k}", tag=f"zt{k}")
              for k in range(8)]
        q = b_sz // 4
        for h in range(4):
            csl = slice(q * h, q * (h + 1))
            for k in range(8):
                nc.gpsimd.dma_start(zt[k][:, csl],
                                    z_tm[128 * k:128 * (k + 1), csl])

        # ---- carry chains: fixed width-512 chain groups, decoupled
        # from the (tapering) output groups so the tail groups do not
        # re-run chain matmuls ----
        CHW = 512
        n_cg = b_sz // CHW
        ktg = [[None] * NCH for _ in range(n_cg)]
        for cg in range(n_cg):
            ktg[cg][0] = p0sb[:, CHW * cg:CHW * (cg + 1)]

        def emit_chain_step(cg, m):
            # kt[cg][m+1] from kt[cg][m] + chunk-m z; the ACT copy
            # overlaps the surrounding main matmuls
            ms = min(m, 1)
            csl = slice(CHW * cg, CHW * (cg + 1))
            cps = ps_c.tile([4, CHW], f32, tag="cps")
            nc.tensor.matmul(cps[:], bwv(ms, 0), zt[2 * m][:, csl],
                             start=True, stop=False)
            nc.tensor.matmul(cps[:], bwv(ms, 1), zt[2 * m + 1][:, csl],
                             start=False, stop=False)
            nc.tensor.matmul(cps[:], mwm(ms), ktg[cg][m],
                             start=False, stop=True)
            ktt = ktp.tile([4, CHW], bf, name=f"kt_{cg}_{m + 1}",
                           tag=f"kt{cg}_{m + 1}")
            nc.scalar.copy(ktt[:], cps[:])
            ktg[cg][m + 1] = ktt[:]

        # ---- main loop over batch groups ----
        for jg, JG in enumerate(GROUPS):
            j0 = int(gstart[jg])
            kw = 128 * JG
            gsl = slice(128 * j0, 128 * j0 + kw)

            if jg in (2, 3):
                warm_burst(8)
            pos_stage = [None] * JG
            vel_stage = [None] * JG
            for m in range(NCH):
                ms = min(m, 1)
                if m + 1 < NCH and jg < n_cg:
                    emit_chain_step(jg, m)

                for jj in range(JG):
                    j = j0 + jj
                    bsl = slice(128 * j, 128 * (j + 1))
                    if m == 0:
                        pos_stage[jj] = stage.tile([128, 1024], f32,
                                                   name=f"pos_st_{j}",
                                                   tag=f"pos_st{jj}")
                        vel_stage[jj] = stage.tile([128, 1024], f32,
                                                   name=f"vel_st_{j}",
                                                   tag=f"vel_st{jj}")
                    out_ps = ps_main.tile([128, 512], f32, tag="out")
                    # both big-N matmuls first so the next stationary
                    # load always hides under a 512-col stream; the
                    # short carry matmul closes the accumulation
                    nc.tensor.matmul(out_ps[:], zt[2 * m][:, bsl],
                                     wmain(ms, 0), start=True, stop=False)
                    nc.tensor.matmul(out_ps[:], zt[2 * m + 1][:, bsl],
                                     wmain(ms, 1), start=False, stop=False)
                    cg = (128 * j) // CHW
                    koff = 128 * j - CHW * cg
                    nc.tensor.matmul(out_ps[:, 0:CW_N],
                                     ktg[cg][m][:, koff:koff + 128],
                                     cwm(ms), start=False, stop=True)

                    csl = slice(256 * m, 256 * (m + 1))
                    nc.vector.tensor_copy(pos_stage[jj][:, csl],
                                          out_ps[:, 0:256])
                    nc.scalar.copy(vel_stage[jj][:, csl], out_ps[:, 256:512])
                    if JG <= 2 and m == 1:
                        nc.sync.dma_start(pos_out[bsl, 0:512],
                                          pos_stage[jj][:, 0:512])
                        nc.sync.dma_start(vel_out[bsl, 0:512],
                                          vel_stage[jj][:, 0:512])
                    if m == NCH - 1:
                        lo = 512 if JG <= 2 else 0
                        nc.sync.dma_start(pos_out[bsl, lo:1024],
                                          pos_stage[jj][:, lo:1024])
                        nc.sync.dma_start(vel_out[bsl, lo:1024],
                                          vel_stage[jj][:, lo:1024])
                    if jg == 0 and m == 0 and jj < 1:
                        warm_burst(6)

    nc.compile()
    return nc


# ----------------------------------------------------------------------------
# Host entry point
# ----------------------------------------------------------------------------

_CACHE = {}

# test-harness knobs (ignored in normal use)
PROFILE = False
LAST_RESULT = None


def _get_nc(n_bt):
    if n_bt not in _CACHE:
        _CACHE[n_bt] = build_nc(n_bt)
    return _CACHE[n_bt]


def kernel(pred_vel, dt, p0, q_pos, q_vel, r_vel):
    from concourse.bass_utils import run_bass_kernel_spmd

    z = np.asarray(pred_vel, dtype=np.float32)
    p0 = np.asarray(p0, dtype=np.float32)
    assert z.shape == (B_FULL, T, 2) and p0.shape == (B_FULL, 2)

    weights = build_weights(dt, q_pos, q_vel, r_vel)
    # initial carry rows (p_c0, p_c1, v_c0, v_c1) = (p0x, p0y, 0, 0)
    p0z = np.zeros((4, B_FULL), dtype=bf16)
    p0z[0] = p0[:, 0].astype(bf16)
    p0z[1] = p0[:, 1].astype(bf16)
    nc = _get_nc(B_CORE // 128)

    in_maps = []
    for i in range(N_CORES):
        sl = slice(i * B_CORE, (i + 1) * B_CORE)
        m = {"z_tm": np.ascontiguousarray(z[sl].reshape(B_CORE, 2 * T).T),
             "p0z_in": np.ascontiguousarray(p0z[:, sl])}
        m.update(weights)
        in_maps.append(m)

    res = run_bass_kernel_spmd(nc, in_maps, core_ids=list(range(N_CORES)),
                               trace=PROFILE)
    global LAST_RESULT
    LAST_RESULT = res
    pos = np.concatenate([r["pos_out"].reshape(B_CORE, T, 2)
                          for r in res.results], axis=0)
    vel = np.concatenate([r["vel_out"].reshape(B_CORE, T, 2)
                          for r in res.results], axis=0)
    return pos, vel
